# revision 25
# baseline (speedup 1.0000x reference)
"""Trainium2 Bass kernel for nn_BatchConv1d (dynamic grouped conv attention).

Reference computation (per batch b):
    kernel = (q @ W_kernel + b_kernel).reshape(Lq, C, KW)      # dynamic conv kernels
    bias   = (q @ W_bias + b_bias)[:, 0]
    kpad   = zero-pad k along L by PAD=1
    a[i,j] = sum_{c,w} kernel[i,c,w] * kpad[j+w,c] + bias[i] + bias_b

Key reassociation: the output is bilinear in q and k, so
    a[i,j] = sum_d q_ext[i,d] * M'[d,j]
where q_ext = [q | 1] (Lq x 513) and
    M'[d,j] = sum_{c,w} W_kernel[d,c,w] * kpad[j+w,c] + W_bias[d]   (d < 512)
    M'[512,j] = r[j] = sum_{c,w} b_kernel[c,w] * kpad[j+w,c] + b_bias + bias_b
This replaces the per-query dynamic conv (1024x1024x1536 MACs) with a
static conv of W with k (512x1024x1536) plus one small GEMM
(1024x1024x513) -- ~1.9x fewer PE cycles -- and all transposes move to
the host (inputs are DMA'd as exact SBUF images).

Per core (data-parallel over B=8, one batch per NeuronCore):
  Stage A (PE): M'[dt][p, j] = sum_{ct,w} WT[ct*3+w][:, dt*128+p] . kT_pad[ct][:, j+w]
     4 d-tiles x 2 j-chunks, 12-matmul PSUM accumulation each; W_bias
     column added during the PSUM->SBUF copy (DVE), which casts to bf16.
  r-row (Pool): 12 fused mult-accumulate ops (acc[p,j] += kT[ct*128+p, j+w]
     * b_kernel[ct*128+p, w], const folded in as +const/128 per partition)
     then partition_all_reduce -> mp tile 4 (every partition = r; stage B's
     lhsT rows 1..127 are zero so only row 0 matters).
  Stage B (PE): out[i, j] = sum_{dt<5} qT_ext[dt][:, i] . M'[dt][:, j]
     (5-matmul accumulation; PSUM->SBUF bf16 copy on DVE/Act, then DMA).
All matmul operands are bf16 (1 cyc/row on the PE); accumulation is fp32
in PSUM. The output travels as bf16 and is upcast to fp32 on the host.
"""

import numpy as np
from contextlib import ExitStack

import ml_dtypes

import concourse.bass as bass
import concourse.bass_isa as bass_isa
import concourse.mybir as mybir
import concourse.tile as tile
from concourse import bacc
from concourse.bass_utils import run_bass_kernel_spmd

F32 = mybir.dt.float32
BF16 = mybir.dt.bfloat16

B, Lq, Lk, D, C, KW = 8, 1024, 1024, 512, 512, 3
CW = C * KW            # 1536
NT_D = D // 128        # 4 stage-A output d-tiles
NT_DE = NT_D + 1       # 5 stage-B contraction tiles (4 q-tiles + bias row)
NT_C = C // 128        # 4
NT_I = Lq // 128       # 8
NT_W = CW // 128       # 12 (tile t = ct*3 + w)
LKP = Lk + 2           # 1026, kT with one zero col each side
WSEC = NT_W * 128      # 1536 cols per wt d-section

_CACHE = {}


def _build(repeats=1):
    nc = bacc.Bacc(target_bir_lowering=False, debug=False)

    # inputs are host-prepared SBUF images: [128 partitions, free] with the
    # exact on-chip column layout, so every DMA moves large contiguous
    # chunks (elem >= 512B avoids the 2x DMA-engine latency penalty)
    kt_in = nc.dram_tensor("kt_in", [128, NT_C * LKP], BF16, kind="ExternalInput").ap()
    wt_in = nc.dram_tensor("wt_in", [128, NT_D * WSEC], BF16, kind="ExternalInput").ap()
    qt_in = nc.dram_tensor("qt_in", [128, NT_DE * Lq], BF16, kind="ExternalInput").ap()
    wb_in = nc.dram_tensor("wb_in", [128, NT_D], F32, kind="ExternalInput").ap()
    bk_in = nc.dram_tensor("bk_in", [128, NT_W], F32, kind="ExternalInput").ap()
    out = nc.dram_tensor("out", [Lq, Lk], BF16, kind="ExternalOutput").ap()

    with tile.TileContext(nc) as tc:
        for rep in range(repeats):
            _emit_body(nc, tc, rep, kt_in, wt_in, qt_in, wb_in, bk_in, out)

    nc.compile()
    return nc


def _emit_body(nc, tc, rep, kt_in, wt_in, qt_in, wb_in, bk_in, out):
    R = f"r{rep}_"
    with ExitStack() as ctx:
        persist = ctx.enter_context(tc.tile_pool(name=R + "persist", bufs=1))
        out_pool = ctx.enter_context(tc.tile_pool(name=R + "outp", bufs=8))

        # mega-tiles so one strided DMA covers many logical tiles (HWDGE has
        # a fixed ~625 ns cost per dma_start; the tile framework tracks
        # sub-tile ranges so partial writes don't false-serialize readers).
        # wt is SECTION-major: wt_all[:, s*1536 + (ct*3+w)*128 + d].
        kt_all = persist.tile([128, NT_C * LKP], BF16, tag="kt")
        wt_all = persist.tile([128, NT_D * WSEC], BF16, tag="wt")
        qt_all = persist.tile([128, NT_DE * Lq], BF16, tag="qt")
        mp_all = persist.tile([128, NT_DE * Lk], BF16, tag="mp")
        wb_sb = persist.tile([128, NT_D], F32, tag="wb")
        bk_sb = persist.tile([128, NT_W], F32, tag="bk")
        racc = persist.tile([128, Lk], F32, tag="racc")

        kt_sb = [kt_all[:, t * LKP:(t + 1) * LKP] for t in range(NT_C)]
        qt_sb = [qt_all[:, t * Lq:(t + 1) * Lq] for t in range(NT_DE)]
        mp_sb = [mp_all[:, t * Lk:(t + 1) * Lk] for t in range(NT_DE)]

        def wt_lhsT(t, dt):
            off = dt * WSEC + t * 128
            return wt_all[:, off:off + 128]

        kt_dst = kt_all[:].rearrange("p (t j) -> p t j", t=NT_C)
        kt_src = kt_in.rearrange("p (t j) -> p t j", t=NT_C)

        # ---- input DMAs, one queue (SP/HWDGE), consumption order -----------
        # stage A jc=0 needs wt d-section 0 + kt cols [0:514] of each tile;
        # the Pool r-row chain needs full kt tiles, so kt jc=1 cols come
        # right after the wt sections; qt is only needed by stage B.
        nc.gpsimd.dma_start(bk_sb[:], bk_in[:])
        nc.gpsimd.dma_start(wb_sb[:], wb_in[:])
        nc.sync.dma_start(wt_all[:, 0:384], wt_in[:, 0:384])
        nc.sync.dma_start(kt_dst[:, 0, 0:514], kt_src[:, 0, 0:514])
        nc.sync.dma_start(kt_dst[:, 0, 514:LKP], kt_src[:, 0, 514:LKP])
        nc.sync.dma_start(wt_all[:, 384:WSEC], wt_in[:, 384:WSEC])
        for ct in range(1, NT_C):
            nc.sync.dma_start(kt_dst[:, ct, :], kt_src[:, ct, :])
        nc.sync.dma_start(wt_all[:, WSEC:2 * WSEC], wt_in[:, WSEC:2 * WSEC])
        nc.sync.dma_start(wt_all[:, 2 * WSEC:3 * WSEC], wt_in[:, 2 * WSEC:3 * WSEC])
        nc.sync.dma_start(wt_all[:, 3 * WSEC:4 * WSEC], wt_in[:, 3 * WSEC:4 * WSEC])
        nc.sync.dma_start(qt_all[:], qt_in[:])

        # ---- r row on Pool: acc[p,j] = sum_(ct,w) kT[ct*128+p, j+w]*bk[...] --
        # partition_all_reduce then writes r to every partition of mp tile 4
        # (only row 0 is picked up by stage B's lhsT; rows 1..127 multiply
        # zeros). The scalar const (b_bias + bias_b) is added on the host.
        first = True
        for ct in range(NT_C):
            for w in range(KW):
                t = ct * KW + w
                src = kt_sb[ct][:, w:w + Lk]
                if first:
                    nc.vector.tensor_scalar(
                        racc[:], src, bk_sb[:, t:t + 1], None,
                        mybir.AluOpType.mult,
                    )
                    first = False
                else:
                    nc.vector.scalar_tensor_tensor(
                        racc[:], src, bk_sb[:, t:t + 1], racc[:],
                        op0=mybir.AluOpType.mult, op1=mybir.AluOpType.add,
                    )
        nc.gpsimd.partition_all_reduce(
            mp_sb[NT_D][:], racc[:], 128, bass_isa.ReduceOp.add,
        )

        psA_ctx = tc.tile_pool(name=R + "psA", bufs=2, space="PSUM")
        psB_ctx = tc.tile_pool(name=R + "psB", bufs=4, space="PSUM")
        psA = psA_ctx.__enter__()
        psB = psB_ctx.__enter__()

        def emit_A(jc, dts):
            for dt in dts:
                ps = psA.tile([128, 512], F32, tag="a", name=R + "a")
                idx = 0
                for ct in range(NT_C):
                    for w in range(KW):
                        nc.tensor.matmul(
                            ps[:],
                            wt_lhsT(ct * KW + w, dt),
                            kt_sb[ct][:, jc * 512 + w:jc * 512 + w + 512],
                            start=(idx == 0),
                            stop=(idx == NT_W - 1),
                        )
                        idx += 1
                nc.scalar.add(
                    mp_sb[dt][:, jc * 512:(jc + 1) * 512],
                    ps[:], wb_sb[:, dt:dt + 1],
                )

        def emit_B(jc, its, last=False):
            for n, it in enumerate(its):
                ps = psB.tile([128, 512], F32, tag="b", name=R + "b")
                is_last = last and n == len(its) - 1
                # DVE-copied tiles fuse the r-row add into the copy
                # (tensor_tensor with mp tile 4) and skip the 5th matmul;
                # Act-copied tiles keep the 5-matmul chain (Act has no
                # tensor_tensor).
                fused = not is_last
                nk = NT_D if fused else NT_DE
                for dt in range(nk):
                    nc.tensor.matmul(
                        ps[:],
                        qt_sb[dt][:, it * 128:(it + 1) * 128],
                        mp_sb[dt][:, jc * 512:(jc + 1) * 512],
                        start=(dt == 0),
                        stop=(dt == nk - 1),
                    )
                o_sb = out_pool.tile([128, 512], BF16, tag="o", name=R + "o")
                orow = out[it * 128:(it + 1) * 128, jc * 512:(jc + 1) * 512]
                if is_last:
                    # split the final tile across both copy engines and both
                    # DMA queues so the kernel tail is half a tile deep
                    nc.vector.tensor_copy(o_sb[:, 0:256], ps[:, 0:256])
                    nc.scalar.copy(o_sb[:, 256:512], ps[:, 256:512])
                    nc.sync.dma_start(orow[:, 0:256], o_sb[:, 0:256])
                    nc.scalar.dma_start(orow[:, 256:512], o_sb[:, 256:512])
                elif fused:
                    nc.vector.tensor_tensor(
                        o_sb[:], ps[:], mp_sb[NT_D][:, jc * 512:(jc + 1) * 512],
                        mybir.AluOpType.add,
                    )
                    nc.sync.dma_start(orow[:], o_sb[:])
                else:
                    nc.scalar.copy(o_sb[:], ps[:])
                    nc.sync.dma_start(orow[:], o_sb[:])

        emit_A(0, range(NT_D))
        emit_A(1, range(NT_D))
        emit_B(0, range(NT_I))
        emit_B(1, range(NT_I), last=True)

        psB_ctx.__exit__(None, None, None)
        psA_ctx.__exit__(None, None, None)


def _get_nc():
    if "nc" not in _CACHE:
        _CACHE["nc"] = _build()
    return _CACHE["nc"]


def _prepare_in_maps(q, k, W_kernel, b_kernel, W_bias, b_bias, bias_b):
    q = np.asarray(q, dtype=np.float32)
    k = np.asarray(k, dtype=np.float32)
    W_kernel = np.asarray(W_kernel, dtype=np.float32)
    b_kernel = np.asarray(b_kernel, dtype=np.float32)
    W_bias = np.asarray(W_bias, dtype=np.float32)
    b_bias = np.asarray(b_bias, dtype=np.float32)
    bias_b = np.asarray(bias_b, dtype=np.float32)
    bf16 = ml_dtypes.bfloat16

    # wt[w*C + c, d] = W_kernel[d, c, w]; SBUF image, section-major with
    # ct-major tile order t = ct*3 + w:
    # wt_img[p, s*1536 + (ct*3+w)*128 + d] = wt[w*512 + ct*128 + p, s*128 + d]
    wt = W_kernel.reshape(D, C, KW).transpose(2, 1, 0).reshape(CW, D)
    wt_img = np.ascontiguousarray(
        wt.reshape(KW, NT_C, 128, NT_D, 128).transpose(2, 3, 1, 0, 4).reshape(128, -1)
    ).astype(bf16)

    # wb columns: per-d-tile scalar added during the M' copy
    wb = np.ascontiguousarray(W_bias[:, 0].reshape(NT_D, 128).T)

    # bk columns for the Pool r-row chain: bk_img[p, ct*3+w] = b_kernel[(ct*128+p)*3 + w]
    bk3 = b_kernel.reshape(C, KW)
    bk_img = np.zeros((128, NT_W), np.float32)
    for ct in range(NT_C):
        for w in range(KW):
            bk_img[:, ct * KW + w] = bk3[ct * 128:(ct + 1) * 128, w]
    rconst = float(b_bias.reshape(-1)[0] + bias_b.reshape(-1)[0])

    in_maps = []
    for b in range(B):
        kt = np.zeros((C, LKP), np.float32)
        kt[:, 1:Lk + 1] = k[b].T
        kt_img = np.ascontiguousarray(
            kt.reshape(NT_C, 128, LKP).transpose(1, 0, 2).reshape(128, -1)
        ).astype(bf16)
        qt = np.zeros((NT_DE * 128, Lq), np.float32)
        qt[:D] = q[b].T
        qt[D] = 1.0
        qt_img = np.ascontiguousarray(
            qt.reshape(NT_DE, 128, Lq).transpose(1, 0, 2).reshape(128, -1)
        ).astype(bf16)
        in_maps.append({
            "kt_in": kt_img,
            "wt_in": wt_img,
            "qt_in": qt_img,
            "wb_in": wb,
            "bk_in": bk_img,
        })
    return in_maps, rconst


def kernel(q, k, W_kernel, b_kernel, W_bias, b_bias, bias_b):
    in_maps, rconst = _prepare_in_maps(
        q, k, W_kernel, b_kernel, W_bias, b_bias, bias_b
    )
    res = run_bass_kernel_spmd(_get_nc(), in_maps, core_ids=list(range(B)))
    return np.stack(
        [res.results[b]["out"].astype(np.float32) + rconst for b in range(B)],
        axis=0,
    )
